# revision 47
# baseline (speedup 1.0000x reference)
"""Causal multi-head attention on 8 TRN2 NeuronCores.

Reference computation (fp32):
    q,k,v = x @ {Q,K,V}.T split into 16 heads of 64
    scores = q k^T / 8, causal mask, softmax
    out    = (attn @ v concat heads) @ W_o.T

Sharding: core c (0..7) takes batch b = c//4 and head group g = c%4
(heads 4g..4g+3, i.e. a 256-row slice of Q/K/V and a 256-column slice
of W_o). Each core produces a partial [T, D] output (bf16); the host
sums the 4 partials per batch in fp32. No on-device collectives.

Per-core DRAM layout (host pre-packs everything so each input lands in
one or two big contiguous DMAs and every matmul contraction dim sits on
SBUF partitions):
    xall  [128, 16384]  x[b].T packed (p, tch, db, c) chunk-major
    wq/wk [128, 2048]   Q/K slice^T packed db-major
    wv    [128, 2080]   V^T with a zero column after each head (the
                        ones-column, added via a rank-1 matmul, makes
                        the PV matmul emit the softmax denominator)
    wo    [128, 2048]   W_o[:, slice].T packed db-major
    maskz [128, 2048]   mask constants; only [:, 0:128] used per block
    ones [1, 128], wv1 [1, 260]

Attention is computed transposed (ST[tk, tq] = k-block . qT-chunk) so
softmax exp is elementwise and PV needs no transposes; exp runs on ACT
straight out of PSUM.  kT weight loads are full 128-row loads (both
heads of the pair stacked); the streamed qT is kept in per-head
zero-padded tiles so the other head's rows contribute nothing.  This
avoids the 64-row tile-position weight loads that serialize the PE
queue on every ST<->PV transition (~95 ns each).  Softmax
normalization runs off the PE path: one [65,CH] bf16 copy drains the
PSUM accumulator (freeing it for the next pair), then denominator row
-> partition-0 tile -> approx reciprocal -> gpsimd partition-broadcast
-> multiply (reciprocal and broadcast only work from partition 0).
Output is bf16 (host sums partials in fp32); stage-5 tail blocks drain
PSUM via ACT's Copy activation once exp work is done.  189.8us
baseline -> 164.9us measured (PE stream floor ~111us at 2.4 GHz).
"""

import numpy as np

import concourse.bass as bass  # noqa: F401
import concourse.tile as tile
from concourse import bacc, mybir
from concourse.bass_utils import run_bass_kernel_spmd



F32 = mybir.dt.float32
F32R = mybir.dt.float32r
BF16 = mybir.dt.bfloat16
EXP = mybir.ActivationFunctionType.Exp
ACOPY = mybir.ActivationFunctionType.Copy

import os as _os

# matmul operand dtype: bf16 (full PE rate + fast weight load; all
# accumulations stay in fp32 PSUM and softmax denominators are computed
# in fp32, so the only loss is bf16 input/intermediate rounding,
# ~5e-3 relative).
WDT = BF16 if _os.environ.get("MHA_DTYPE", "bf16") == "bf16" else F32R

N_CORES = 8
T = 2048          # sequence length
D = 1024          # model dim
HPC = 4           # heads per core
HD = 64           # head dim
DS = HPC * HD     # 256: per-core slice of D
VW = HPC * (HD + 1)  # 260: v tiles with ones-column per head
CH = 1024         # tq chunk width
NCH = T // CH     # chunks
NTB = T // 128    # 128-row t blocks
NDB = D // 128    # 128-row d blocks


def build_program():
    nc = bacc.Bacc("TRN2", target_bir_lowering=False, debug=False,
                   num_devices=N_CORES)
    xall_d = nc.dram_tensor("xall", [128, 4 * NDB * 512], WDT,
                            kind="ExternalInput").ap()
    wq_d = nc.dram_tensor("wq", [128, NDB * DS], WDT,
                          kind="ExternalInput").ap()
    wk_d = nc.dram_tensor("wk", [128, NDB * DS], WDT,
                          kind="ExternalInput").ap()
    wv_d = nc.dram_tensor("wv", [128, NDB * VW], WDT,
                          kind="ExternalInput").ap()
    wo_d = nc.dram_tensor("wo", [128, 2 * D], WDT,
                          kind="ExternalInput").ap()
    maskz_d = nc.dram_tensor("maskz", [128, 2048], WDT,
                             kind="ExternalInput").ap()
    ones_d = nc.dram_tensor("ones", [1, 128], WDT, kind="ExternalInput").ap()
    wv1_d = nc.dram_tensor("wv1", [1, VW], WDT, kind="ExternalInput").ap()
    out_d = nc.dram_tensor("out", [T, D], BF16, kind="ExternalOutput").ap()

    XCW = NDB * 512  # 4096: packed x columns per tq chunk

    with tile.TileContext(nc) as tc, \
         tc.tile_pool(name="xt", bufs=1) as xt_pool, \
         tc.tile_pool(name="ww", bufs=1) as ww_pool, \
         tc.tile_pool(name="cst", bufs=1) as cst_pool, \
         tc.tile_pool(name="qp", bufs=16) as qp_pool, \
         tc.tile_pool(name="kp", bufs=8) as kp_pool, \
         tc.tile_pool(name="vv", bufs=16) as vv_pool, \
         tc.tile_pool(name="ot", bufs=4) as ot_pool, \
         tc.tile_pool(name="ee", bufs=4) as e_pool, \
         tc.tile_pool(name="ou", bufs=2) as ou_pool, \
         tc.tile_pool(name="rd", bufs=2) as rd_pool, \
         tc.tile_pool(name="rb", bufs=2) as rb_pool, \
         tc.tile_pool(name="ob", bufs=3) as ob_pool:

        # ---- big input DMAs ------------------------------------------
        # chunk 0, wq split fine so the first projection matmuls unblock
        # early, and concurrent DMA instructions engage more DMA engines
        # (one instruction only sustains ~50-120 GB/s of the 360 GB/s
        # aggregate).  wq rides the SP queue because ACT's queue is
        # blocked by its act-table load for the first ~1.3us; the tiny
        # ones/wv1 constants come after the first x/w wave (each issue
        # costs ~0.7us of queue time and v groups start late anyway).
        xt0_t = [xt_pool.tile([128, 1024], WDT, tag="xt0", name=f"xt0_{i}",
                              bufs=4)
                 for i in range(4)]
        xtall = [None] + [xt_pool.tile([128, XCW], WDT, tag="xt",
                                       name=f"xt{tch}", bufs=2)
                          for tch in range(1, 3)] + [None]
        wq_t = [ww_pool.tile([128, 1024], WDT, tag="wq", name=f"wq{i}",
                             bufs=2)
                for i in range(2)]
        wk_t = [ww_pool.tile([128, 1024], WDT, tag="wk", name=f"wk{i}",
                             bufs=2)
                for i in range(2)]
        wvt_t = [ww_pool.tile([128, 4 * VW], WDT, tag="wvt", name=f"wv{i}",
                              bufs=2)
                 for i in range(2)]
        nc.sync.dma_start(wq_t[0][:], wq_d[:, 0:1024])
        nc.sync.dma_start(xt0_t[0][:], xall_d[:, 0:1024])
        nc.sync.dma_start(xt0_t[1][:], xall_d[:, 1024:2048])
        nc.scalar.dma_start(wq_t[1][:], wq_d[:, 1024:2048])
        nc.sync.dma_start(xt0_t[2][:], xall_d[:, 2048:3072])
        nc.sync.dma_start(xt0_t[3][:], xall_d[:, 3072:4096])
        nc.sync.dma_start(wk_t[0][:], wk_d[:, 0:1024])
        nc.scalar.dma_start(wk_t[1][:], wk_d[:, 1024:2048])
        ones_t = cst_pool.tile([1, 128], WDT, tag="ones")
        nc.sync.dma_start(ones_t[:], ones_d[:])
        wv1_t = cst_pool.tile([1, VW], WDT, tag="wv1")
        nc.sync.dma_start(wv1_t[:], wv1_d[:])
        for i in range(2):
            nc.scalar.dma_start(wvt_t[i][:],
                                wv_d[:, 4 * VW * i:4 * VW * i + 4 * VW])
        nc.sync.dma_start(xtall[1][:], xall_d[:, XCW:2 * XCW])
        # only the first 128 mask columns (the in-block triangle) are used
        maskz_t = cst_pool.tile([128, 128], WDT, tag="maskz")
        nc.scalar.dma_start(maskz_t[:], maskz_d[:, 0:128])
        wo_t = ww_pool.tile([128, 2 * D], WDT, tag="wo")
        nc.scalar.dma_start(wo_t[:], wo_d[:])
        nc.sync.dma_start(xtall[2][:], xall_d[:, 2 * XCW:3 * XCW])
        # chunk 3 recycles chunk 0's four buffers (tag xt0): chunk 0 is
        # fully consumed by ~35us, chunk 3 first read by fillers ~60us in
        xt3_t = [xt_pool.tile([128, 1024], WDT, tag="xt0", name=f"xt3_{i}",
                              bufs=4)
                 for i in range(4)]
        for i in range(4):
            nc.sync.dma_start(xt3_t[i][:],
                              xall_d[:, 3 * XCW + 1024 * i:
                                     3 * XCW + 1024 * i + 1024])

        def xsl(tch, db):
            # (tile, col0) for the [128, 512] x block of (tq chunk, db)
            if tch == 0:
                return xt0_t[db // 2], 512 * (db % 2)
            if tch == 3:
                return xt3_t[db // 2], 512 * (db % 2)
            return xtall[tch], 512 * db

        # ---- persistent padded qT tiles: head h data in rows
        # 64*(h%2)..+64, zeros elsewhere (so a full 128-row kT stationary
        # computes only head h's scores when streaming this tile).
        qp_t = [[qp_pool.tile([128, 512], WDT, tag="qp",
                              name=f"qp{h}_{tch}")
                 for tch in range(4)] for h in range(HPC)]
        for tch in range(4):
            for h in range(HPC):
                zr = 64 * ((h + 1) % 2)
                nc.gpsimd.memset(qp_t[h][tch][zr:zr + 64, :], 0)

        kT_t = [[None] * 4 for _ in range(2)]
        v_t = [None] * NTB

        # ---- persistent E tiles (PV only reads exp-written regions) ----
        e_tiles = [e_pool.tile([128, CH], WDT, tag="ee", name=f"ee{i}")
                   for i in range(4)]

        # oT_t[db][c]: [128, CH] attention outputs, d on partitions
        # (head h lives in tile h//2 rows 64*(h%2)..+64)
        oT_t = [[ot_pool.tile([128, CH], WDT, tag="ot", name=f"ot{d}_{c}")
                 for c in range(NCH)] for d in range(2)]
        state = {"eidx": 0}

        with tc.tile_pool(name="pst", bufs=2, space="PSUM") as pst_pool, \
             tc.tile_pool(name="pac", bufs=1, space="PSUM") as pac_pool:

            def emit_qk_group(ps1_pool, tch, eb, wt):
                ps = ps1_pool.tile([128, 512], F32, tag="ps1",
                                   name=f"p1_{tch}_{eb}_{wt is wk_t}")
                for db in range(NDB):
                    xt, c0 = xsl(tch, db)
                    nc.tensor.matmul(
                        ps[:],
                        wt[db // 4][:, DS * (db % 4) + 128 * eb:
                                    DS * (db % 4) + 128 * eb + 128],
                        xt[:, c0:c0 + 512],
                        start=(db == 0), stop=(db == NDB - 1))
                if wt is wq_t:
                    for par in range(2):
                        nc.vector.tensor_copy(
                            qp_t[2 * eb + par][tch][64 * par:64 * par + 64, :],
                            ps[64 * par:64 * par + 64, :])
                else:
                    k = kp_pool.tile([128, 512], WDT, tag="kp",
                                     name=f"kp_{tch}_{eb}")
                    nc.vector.tensor_copy(k[:], ps[:])
                    kT_t[eb][tch] = k

            def emit_v_group(ps1_pool, tb):
                tch, tsub = tb // 4, 128 * (tb % 4)
                ps = ps1_pool.tile([128, VW], F32, tag="ps1",
                                   name=f"p1v_{tb}")
                for db in range(NDB):
                    xt, c0 = xsl(tch, db)
                    nc.tensor.matmul(
                        ps[:],
                        xt[:, c0 + tsub:c0 + tsub + 128],
                        wvt_t[db // 4][:, VW * (db % 4):VW * (db % 4) + VW],
                        start=(db == 0), stop=False)
                # ones-columns: rank-1 update 1s^T . wv1
                nc.tensor.matmul(ps[:], ones_t[:], wv1_t[:],
                                 start=False, stop=True)
                v = vv_pool.tile([128, VW], WDT, tag="vv", name=f"v{tb}")
                nc.vector.tensor_copy(v[:], ps[:])
                v_t[tb] = v

            def emit_stage1_tch(ps1_pool, tch):
                for wt in (wq_t, wk_t):
                    for eb in range(2):
                        emit_qk_group(ps1_pool, tch, eb, wt)
                for tb in range(4 * tch, 4 * tch + 4):
                    emit_v_group(ps1_pool, tb)

            def emit_pv(acc, c, h, j, e, off):
                jmax = 8 * c + 7
                alg = (off // 512) * 512
                for s in range(alg, CH, 512):
                    lo = max(s, off)
                    nc.tensor.matmul(
                        acc[:, lo:s + 512],
                        v_t[j][:, 65 * h:65 * h + 65],
                        e[:, lo:s + 512],
                        start=(j == 0),
                        stop=(j == (8 * c + 3 if s == 0 else jmax)),
                    )

            def emit_pair(c, h, fine=False):
                # attention for one (chunk, head), PV delayed 2 j-steps so
                # the exp (ACT) latency never stalls the PE stream.
                # fine=True runs the normalization in 256-col sub-chains
                # so dependents (the stage-5 tail) unlock column-range by
                # column-range instead of waiting for the whole chain.
                pb, rw = h // 2, 64 * (h % 2)
                jmax = 8 * c + 7
                acc = pac_pool.tile([65, CH], F32, tag="pac",
                                    name=f"ac{c}_{h}")
                pending = []
                for j in range(jmax + 1):
                    off = max(0, 128 * j - CH * c)
                    alg = (off // 512) * 512  # 512-aligned ST psum base
                    st = pst_pool.tile([128, CH], F32, tag="pst",
                                       name=f"st{c}_{h}_{j}")
                    # ST[tk, tq] = k-block . qT-chunk; full 128-row
                    # stationary (both heads), per-head zero-padded qT
                    for s in range(alg, CH, 512):
                        lo = max(s, off)
                        nc.tensor.matmul(
                            st[:, lo:s + 512],
                            kT_t[pb][j // 4][:,
                                             128 * (j % 4):128 * (j % 4) + 128],
                            qp_t[h][2 * c + s // 512][:, lo - s:512],
                            start=True, stop=True)
                    e = e_tiles[state["eidx"] % len(e_tiles)]
                    state["eidx"] += 1
                    nc.scalar.activation(e[:, off:], st[:, off:], EXP,
                                         scale=0.125)
                    if 128 * j >= CH * c:
                        # diagonal block: tri mask (PV reads from off on,
                        # so below-diagonal cols never need zeroing)
                        nc.vector.tensor_mul(
                            e[:, off:off + 128], e[:, off:off + 128],
                            maskz_t[:, 0:128])
                    pending.append((j, e, off))
                    if len(pending) > 2:
                        jd, ed, ad = pending.pop(0)
                        emit_pv(acc, c, h, jd, ed, ad)
                for jd, ed, ad in pending:
                    emit_pv(acc, c, h, jd, ed, ad)
                # normalization, off the PE critical path.  The PSUM
                # accumulator is drained to SBUF first so the next pair's
                # PV can recycle the PSUM bank without waiting for the
                # rest of the chain; the denominator row goes straight to
                # a partition-0 tile (reciprocal_approx_fast and
                # partition_broadcast only work from partition 0), then
                # approx-reciprocal, broadcast, multiply.
                oTu = ou_pool.tile([65, CH], BF16, tag="ou")
                for s in range(0, CH, 512):
                    nc.vector.tensor_copy(oTu[:, s:s + 512],
                                          acc[:, s:s + 512])
                step = 256 if fine else 512
                for s in range(0, CH, step):
                    den = rd_pool.tile([1, step], F32, tag="dn",
                                       name=f"dn{c}_{h}_{s}")
                    rden = rd_pool.tile([1, step], F32, tag="rd",
                                        name=f"rd{c}_{h}_{s}")
                    rbt = rb_pool.tile([128, step], F32, tag="rb",
                                       name=f"rb{c}_{h}_{s}")
                    nc.vector.tensor_copy(den[:], oTu[64:65, s:s + step])
                    nc.vector.reciprocal_approx_fast(rden[:], den[:])
                    nc.gpsimd.partition_broadcast(rbt[:], rden[:])
                    nc.vector.tensor_mul(
                        oT_t[pb][c][rw:rw + 64, s:s + step],
                        oTu[0:64, s:s + step], rbt[0:64, :])

            with tc.tile_pool(name="ps1", bufs=2, space="PSUM") as ps1_pool:
                emit_stage1_tch(ps1_pool, 0)
                emit_stage1_tch(ps1_pool, 1)
                # c=0 attention interleaved with the rest of stage 1:
                # stage-1 matmul groups keep the PE dense while ACT
                # works through the exp stream
                fillers = ([("qk", 2, eb, wt)
                            for wt in (wq_t, wk_t)
                            for eb in range(2)]
                           + [("v", tb) for tb in range(8, 12)]
                           + [("qk", 3, eb, wt)
                              for wt in (wq_t, wk_t)
                              for eb in range(2)]
                           + [("v", tb) for tb in range(12, 16)])
                for h in range(HPC):
                    emit_pair(0, h)
                    for f in fillers[4 * h:4 * h + 4]:
                        if f[0] == "qk":
                            emit_qk_group(ps1_pool, f[1], f[2], f[3])
                        else:
                            emit_v_group(ps1_pool, f[1])

            def emit_stage5(ps5_pool, tb, on_act=False):
                # on_act: drain PSUM via the ACT engine's Copy activation
                # (same act table as Exp, no reload) — used for the tail
                # blocks where ACT has run out of exp work but DVE is busy
                c, tw = tb // 8, 128 * (tb % 8)
                pss = [ps5_pool.tile([128, 512], F32, tag="ps5",
                                     name=f"ps5_{tb}_{eb}")
                       for eb in range(2)]
                for db in range(2):
                    for eb in range(2):
                        nc.tensor.matmul(
                            pss[eb][:], oT_t[db][c][:, tw:tw + 128],
                            wo_t[:, D * db + 512 * eb:D * db + 512 * eb + 512],
                            start=(db == 0), stop=(db == 1))
                ob = ob_pool.tile([128, D], BF16, tag="ob")
                for eb in range(2):
                    if on_act:
                        nc.scalar.activation(ob[:, 512 * eb:512 * eb + 512],
                                             pss[eb][:], ACOPY, scale=1.0)
                    else:
                        nc.vector.tensor_copy(ob[:, 512 * eb:512 * eb + 512],
                                              pss[eb][:])
                nc.sync.dma_start(
                    out_d[128 * tb:128 * tb + 128, :], ob[:])

            with tc.tile_pool(name="ps5", bufs=2, space="PSUM") as ps5_pool:
                # c=1 attention interleaved with stage-5 on the finished
                # c=0 chunk; tb 5,6,7 held back so the last pair's
                # normalization chain overlaps PE work
                for h in range(HPC):
                    emit_pair(1, h, fine=(h == 3))
                    if h < 2:
                        emit_stage5(ps5_pool, 2 * h)
                        emit_stage5(ps5_pool, 2 * h + 1)
                    elif h == 2:
                        emit_stage5(ps5_pool, 4)
                for tb in range(5, 8):
                    emit_stage5(ps5_pool, tb, on_act=True)
                for tb in range(8, NTB):
                    emit_stage5(ps5_pool, tb, on_act=(tb % 2 == 0))

    nc.compile()
    return nc


_PROG = None


def _get_prog():
    global _PROG
    if _PROG is None:
        _PROG = build_program()
    return _PROG


def make_in_maps(x, Q, K, V, W_o):
    np_dt = mybir.dt.np(WDT)
    B = x.shape[0]
    maskz = np.zeros((128, 2048), dtype=np.float32)
    for k in range(4):
        blk = maskz[:, 512 * k:512 * k + 512]
        blk[:, 128 * k + 128:] = 1.0
        blk[:, 128 * k:128 * k + 128] = np.greater_equal(
            np.arange(128)[None, :], np.arange(128)[:, None])
    maskz = maskz.astype(np_dt)
    ones = np.ones((1, 128), dtype=np_dt)
    wv1 = np.zeros((1, VW), dtype=np.float32)
    wv1[0, 64::65] = 1.0
    wv1 = wv1.astype(np_dt)

    def pack_db(wT, cols):
        # [D, cols] -> [128, NDB*cols] db-major: out[p, cols*db + c]
        return np.ascontiguousarray(
            wT.reshape(NDB, 128, cols).transpose(1, 0, 2).reshape(
                128, NDB * cols))

    in_maps = []
    for c in range(N_CORES):
        b, g = divmod(c, N_CORES // B)
        sl = slice(DS * g, DS * g + DS)
        wvT = V[sl, :].T  # [D, 256]
        wvT_pad = np.zeros((D, VW), dtype=np.float32)
        for h in range(HPC):
            wvT_pad[:, 65 * h:65 * h + 64] = wvT[:, 64 * h:64 * h + 64]
        xT = np.ascontiguousarray(x[b].T)  # [D, T]
        # pack (p, tch, db, c): xall[p, 4096*tch + 512*db + cc]
        xall = xT.reshape(NDB, 128, 4, 512).transpose(1, 2, 0, 3).reshape(
            128, 4 * NDB * 512)
        woT = np.ascontiguousarray(W_o[:, sl].T)  # [256, 1024]
        wo = woT.reshape(2, 128, D).transpose(1, 0, 2).reshape(128, 2 * D)
        in_maps.append({
            "xall": np.ascontiguousarray(xall).astype(np_dt),
            "wq": pack_db(Q[sl, :].T, DS).astype(np_dt),
            "wk": pack_db(K[sl, :].T, DS).astype(np_dt),
            "wv": pack_db(wvT_pad, VW).astype(np_dt),
            "wo": np.ascontiguousarray(wo).astype(np_dt),
            "maskz": maskz,
            "ones": ones,
            "wv1": wv1,
        })
    return in_maps


def kernel(x, Q, K, V, W_o):
    x = np.asarray(x, dtype=np.float32)
    Q = np.asarray(Q, dtype=np.float32)
    K = np.asarray(K, dtype=np.float32)
    V = np.asarray(V, dtype=np.float32)
    W_o = np.asarray(W_o, dtype=np.float32)

    nc = _get_prog()
    in_maps = make_in_maps(x, Q, K, V, W_o)
    res = run_bass_kernel_spmd(nc, in_maps, core_ids=list(range(N_CORES)))

    B = x.shape[0]
    out = np.zeros((B, T, D), dtype=np.float32)
    for c in range(N_CORES):
        out[c // (N_CORES // B)] += np.asarray(
            res.results[c]["out"]).astype(np.float32)
    return out
